# revision 50
# baseline (speedup 1.0000x reference)
"""PolyCntSketch (TensorSketch, degree 3) Trainium2 kernel.

Math: for each degree d, CountSketch_d = X @ S_d (S_d one-hot signed), then
out = irfft(prod_d rfft(CountSketch_d)).

Device strategy (pure data parallelism over batch, 8 cores, B_core = 1024):
  - Host feeds X transposed ([F, B_core]) in fp16, features packed into
    128-row chunks where each chunk holds whole (block_d0, block_d1,
    block_d2)-classes (block = idx_d // 128), so each chunk touches few
    128-bucket blocks per degree -> few segment matmuls.
  - Stage 1 (weight-stationary, full batch): per (degree, block) the plan's
    Z one-hot matrices accumulate sketch PSUM [128, 512] x 2 half-batches.
    PSUM drains to fp16 SBUF via scalar/gpsimd/vector round-robin.
  - Stage 2: rfft as DFT matmul (512 -> 257 complex), fp16 weights. The
    Nyquist bin Re(256) rides in the identically-zero Im(0) column of the
    DFT matrix (patched to the alternating +-1 column), so it needs no
    extra matmuls; the DC and Nyquist rows of the complex product are
    fixed up with [1, 512] ops at the end of each degree chain.
  - Stage 3: complex product across the 3 degrees, all fp16 (2x DVE rate).
    DFT is scaled by 1/32 (exact) so the fp16 product cannot overflow;
    the irfft table is scaled by 2^15 to compensate.
  - Stage 4: irfft as matmul -> out^T [512, B_core] f32.
"""
import sys

for _p in ("/opt/trn_rl_repo",):
    if _p not in sys.path:
        sys.path.append(_p)

import numpy as np

from concourse import bacc, mybir, tile
from concourse import bass_utils

F16 = mybir.dt.float16
F32 = mybir.dt.float32

B, F, NCOMP, DEG = 8192, 4096, 512, 3
NCORES = 8
B_CORE = B // NCORES
BT = 512                     # batch columns per matmul (PSUM bank = 512 f32)
NBT = B_CORE // BT           # 2 half-batches
CHUNK = 128
NBLK = NCOMP // 128          # 4 bucket blocks
DFT_SCALE = 1.0 / 32.0       # exact power of two; keeps fp16 products small


def _pack_classes(kvs_counts, seed_count=8, iters=60000):
    """Pack (g0,g1,g2)-classes into <=128-row bins, minimizing the total
    matmul count sum_bins sum_d #distinct-blocks. Greedy first-fit by
    marginal cost + hill climbing with move/swap steps."""
    import random

    def cost_of(binkvs):
        if not binkvs:
            return 0
        return sum(len(set((kv >> sh) & 3 for kv in binkvs))
                   for sh in (4, 2, 0))

    items = []
    for kv, s in kvs_counts:
        while s > CHUNK:
            items.append((kv, CHUNK)); s -= CHUNK
        if s:
            items.append((kv, s))

    def greedy(order_classes):
        bins, sizes = [], []
        for kv, s in order_classes:
            best, bestdelta = None, None
            for i, b in enumerate(bins):
                if sizes[i] + s <= CHUNK:
                    delta = (cost_of([k for k, _ in b] + [kv])
                             - cost_of([k for k, _ in b]))
                    if bestdelta is None or delta < bestdelta:
                        best, bestdelta = i, delta
            if best is not None and bestdelta <= 1:
                bins[best].append((kv, s)); sizes[best] += s
            else:
                bins.append([(kv, s)]); sizes.append(s)
        return bins, sizes

    def hill(bins, sizes, seed):
        rng = random.Random(seed)
        bins = [list(b) for b in bins]; sizes = list(sizes)

        def bc(i):
            return cost_of([k for k, _ in bins[i]])

        for _ in range(iters):
            r = rng.random()
            i = rng.randrange(len(bins)); j = rng.randrange(len(bins))
            if i == j or not bins[i]:
                continue
            if r < 0.6:
                ii = rng.randrange(len(bins[i])); kv, s = bins[i][ii]
                if sizes[j] + s > CHUNK:
                    continue
                cb = bc(i) + bc(j)
                bi2 = [x for xi, x in enumerate(bins[i]) if xi != ii]
                ca = (cost_of([k for k, _ in bi2])
                      + cost_of([k for k, _ in bins[j]] + [kv]))
                if ca <= cb:
                    bins[i].pop(ii); bins[j].append((kv, s))
                    sizes[i] -= s; sizes[j] += s
            else:
                if not bins[j]:
                    continue
                ii = rng.randrange(len(bins[i])); jj = rng.randrange(len(bins[j]))
                kv1, s1 = bins[i][ii]; kv2, s2 = bins[j][jj]
                if sizes[i] - s1 + s2 > CHUNK or sizes[j] - s2 + s1 > CHUNK:
                    continue
                cb = bc(i) + bc(j)
                bi2 = [x for xi, x in enumerate(bins[i]) if xi != ii] + [(kv2, s2)]
                bj2 = [x for xj, x in enumerate(bins[j]) if xj != jj] + [(kv1, s1)]
                ca = (cost_of([k for k, _ in bi2])
                      + cost_of([k for k, _ in bj2]))
                if ca <= cb:
                    bins[i][ii] = (kv2, s2); bins[j][jj] = (kv1, s1)
                    sizes[i] += s2 - s1; sizes[j] += s1 - s2
        bins = [b for b in bins if b]
        return bins, sum(cost_of([k for k, _ in b]) for b in bins)

    best = None
    for seed in range(seed_count):
        o = items[:]
        random.Random(seed).shuffle(o)
        if seed % 2 == 0:
            o.sort(key=lambda x: (x[0] >> 2,))
        bins, sizes = greedy(o)
        bins, c = hill(bins, sizes, seed)
        if best is None or c < best[0]:
            best = (c, [list(b) for b in bins])
    return best[1]


def build_plan(index_hash, bit_hash):
    """Pack whole (g0,g1,g2)-classes into 128-row chunks minimizing the
    count of per-(chunk, degree, block) matmuls.

    Returns:
      order [F]: feature order for the transposed X upload
      chunks: list of (start, fill) row ranges into the ordered X
      plan[d][g]: list of (chunk_idx, zslot) in emission order ((d,g)-major)
      zm_t [128, nmm, 128]: stacked Z matrices, partition-major
    """
    idx = np.asarray(index_hash)
    sgn = (np.asarray(bit_hash) * 2 - 1).astype(np.float32)
    blocks = idx >> 7
    key = blocks[0] * 16 + blocks[1] * 4 + blocks[2]

    kvs, counts = np.unique(key, return_counts=True)
    bins = _pack_classes(sorted(zip(kvs.tolist(), counts.tolist())))

    # features per class, consumed front-to-back as bins reference (possibly
    # split) classes
    feat_of = {int(kv): np.nonzero(key == kv)[0].tolist() for kv in kvs}
    order = []
    chunks = []
    for b in bins:
        start = len(order)
        for kv, s in b:
            take = feat_of[kv][:s]
            feat_of[kv] = feat_of[kv][s:]
            order.extend(take)
        chunks.append((start, len(order) - start))
    order = np.array(order)
    assert len(order) == F and len(np.unique(order)) == F

    items = [[[] for _ in range(NBLK)] for _ in range(DEG)]
    for ci, (start, fill) in enumerate(chunks):
        feats = order[start:start + fill]
        for d in range(DEG):
            for g in np.unique(blocks[d, feats]):
                g = int(g)
                rows = np.nonzero(blocks[d, feats] == g)[0]
                Z = np.zeros((CHUNK, 128), np.float32)
                Z[rows, idx[d, feats[rows]] - 128 * g] = sgn[d, feats[rows]]
                items[d][g].append((ci, Z))
    for d in range(DEG):
        for g in range(NBLK):
            if not items[d][g]:
                items[d][g].append((0, np.zeros((CHUNK, 128), np.float32)))

    zmats = []
    plan = [[[] for _ in range(NBLK)] for _ in range(DEG)]
    for d in range(DEG):
        for g in range(NBLK):
            for (ci, Z) in sorted(items[d][g], key=lambda x: x[0]):
                plan[d][g].append((ci, len(zmats)))
                zmats.append(Z)

    # permute chunks into first-use order of the stage-1 quad schedule so
    # consumption-ordered super-tile DMAs read contiguous DRAM rows
    nch = len(chunks)
    seen = set()
    use = []
    for gpair in ((0, 1), (2, 3)):
        n = max(len(plan[d][g]) for d in (0, 1) for g in gpair)
        for i in range(n):
            for g in gpair:
                for d in (0, 1):
                    if i < len(plan[d][g]):
                        ci = plan[d][g][i][0]
                        if ci not in seen:
                            seen.add(ci)
                            use.append(ci)
    for ci in range(nch):
        if ci not in seen:
            use.append(ci)
    old2new = {old: new for new, old in enumerate(use)}
    chunks = [chunks[old] for old in use]
    for d in range(DEG):
        for g in range(NBLK):
            plan[d][g] = [(old2new[ci], zi) for (ci, zi) in plan[d][g]]

    zm = np.stack(zmats)                                # [nmm, 128, 128]
    zm_t = np.ascontiguousarray(zm.transpose(1, 0, 2))  # [128, nmm, 128]
    return order, chunks, plan, zm_t


def build_dft_tables():
    n = np.arange(NCOMP)[:, None]
    k = np.arange(257)[None, :]
    ang = 2 * np.pi * n * k / NCOMP
    # stage-2 lhsT [512, 514]: cols 0..256 Re coeffs, cols 257..513 Im coeffs.
    # Col 257 is Im(0) == 0: replace it with the Nyquist column (-1)^n so
    # Re(256) rides in the Im(0) slot for free.
    dft = np.concatenate([np.cos(ang), -np.sin(ang)], axis=1)
    dft[:, 257] = np.cos(np.pi * np.arange(NCOMP))
    dft = (dft * DFT_SCALE).astype(np.float32)
    dft_t = np.ascontiguousarray(
        dft.reshape(4, 128, 514).transpose(1, 0, 2))    # [128, 4, 514]

    kk = np.arange(257)[:, None]
    nn = np.arange(NCOMP)[None, :]
    ang2 = 2 * np.pi * kk * nn / NCOMP
    ck = np.full((257, 1), 2.0, np.float32); ck[0] = 1.0
    dk = np.full((257, 1), 2.0, np.float32); dk[0] = 0.0
    iscale = 1.0 / DFT_SCALE ** 3
    ire = (ck * np.cos(ang2) / NCOMP) * iscale           # [257, 512]
    iim = (-dk * np.sin(ang2) / NCOMP) * iscale
    ico = np.zeros((4, 128, NCOMP), np.float32)
    ico[0] = ire[0:128]
    ico[1] = ire[128:256]
    ico[2] = iim[0:128]
    ico[3] = iim[128:256]
    # Nyquist rides in the Im(0) slot; its irfft column is (1/N)(-1)^n.
    ico[2, 0] = np.cos(np.pi * np.arange(NCOMP)) / NCOMP * iscale
    ico_t = np.ascontiguousarray(ico.transpose(1, 0, 2))   # [128, 4, 512]
    return dft_t, ico_t


def build_program(plan, chunks, nmm, b_core=B_CORE):
    nch = len(chunks)
    fills = [f for (_, f) in chunks]
    # (d, g) -> [lo, hi) slice into the z stack
    zoff = {}
    pos = 0
    for d in range(DEG):
        for g in range(NBLK):
            zoff[(d, g)] = (pos, pos + len(plan[d][g]))
            pos += len(plan[d][g])
    assert pos == nmm

    nc = bacc.Bacc("TRN2", target_bir_lowering=False, debug=False)
    xp = nc.dram_tensor("xp", [nch * 128, b_core], F16,
                        kind="ExternalInput").ap()
    zm = nc.dram_tensor("zm", [128, nmm, 128], F16, kind="ExternalInput").ap()
    dft = nc.dram_tensor("dft", [128, 4, 514], F16, kind="ExternalInput").ap()
    ico = nc.dram_tensor("ico", [128, 4, 512], F16, kind="ExternalInput").ap()
    ot = nc.dram_tensor("ot", [NCOMP, b_core], F32, kind="ExternalOutput").ap()

    with tile.TileContext(nc) as tc:
        with (
            tc.tile_pool(name="pz", bufs=1) as pz,
            tc.tile_pool(name="pc", bufs=1) as pc,
            tc.tile_pool(name="px", bufs=1) as px,
            tc.tile_pool(name="psk", bufs=1) as psk,
            tc.tile_pool(name="pprod", bufs=1) as pprod,
            tc.tile_pool(name="pfr", bufs=2) as pfr,
            tc.tile_pool(name="ptmp", bufs=2) as ptmp,
            tc.tile_pool(name="prow", bufs=2) as prow,
            tc.tile_pool(name="pout", bufs=2) as pout,
            tc.tile_pool(name="ps_sk", bufs=2, space="PSUM") as ps_sk,
            tc.tile_pool(name="ps_fr", bufs=2, space="PSUM") as ps_fr,
        ):
            # ---- DMA: each dma_start costs ~0.6us of issue time on the
            # sync sequencer, so batch aggressively: X chunks load in
            # consumption-ordered super-tiles (up to 4 chunks = 1MB per
            # issue; the first ones smaller so PE starts ASAP), and the z
            # stacks merge to one issue per (degree, group-pair).
            xsup = {}    # ci -> (tile, slot)
            zts = {}     # (d, g) -> (tile, item offset)

            def load_x(ci):
                xt = px.tile([128, 1, b_core], F16, tag=f"x{ci}",
                             name=f"x{ci}")
                nc.sync.dma_start(xt[:, 0, :], xp[128 * ci:128 * (ci + 1), :])
                xsup[ci] = (xt, 0)

            def load_zmerge(d, glo, ghi, name, split_first=False):
                lo = zoff[(d, glo)][0]
                hi = zoff[(d, ghi)][1]
                zt = pz.tile([128, hi - lo, 128], F16, tag=name, name=name)
                for g in range(glo, ghi + 1):
                    zts[(d, g)] = (zt, zoff[(d, g)][0] - lo)
                if split_first:
                    nc.scalar.dma_start(zt[:, 0:1, :], zm[:, lo:lo + 1, :])
                    nc.scalar.dma_start(zt[:, 1:, :], zm[:, lo + 1:hi, :])
                else:
                    nc.scalar.dma_start(zt[:], zm[:, lo:hi, :])

            # z tables issue on the ACT queue, X super-tiles on the sync
            # queue: two parallel issue pipes (~0.6us per dma_start issue)
            load_zmerge(0, 0, 0, "z00", split_first=True)
            load_zmerge(1, 0, 0, "z10", split_first=True)
            load_zmerge(0, 1, 1, "z01", split_first=True)
            load_zmerge(1, 1, 1, "z11", split_first=True)
            load_zmerge(0, 2, 3, "zB0")
            load_zmerge(1, 2, 3, "zB1")
            load_zmerge(2, 0, 3, "zC")
            for ci in range(min(30, nch)):
                load_x(ci)
            dftt = pc.tile([128, 4, 514], F16, tag="dftt")
            nc.sync.dma_start(dftt[:], dft[:])
            for ci in range(30, nch):
                load_x(ci)
            icot = pc.tile([128, 4, 512], F16, tag="icot")
            nc.sync.dma_start(icot[:], ico[:])

            # ---- stage 1: weight-stationary count sketch over the full
            # batch; the two batch-halves interleave per item so consecutive
            # matmuls alternate PSUM banks (same-bank back-to-back
            # accumulation stalls PE).
            sks = {}

            def s1_drain(d, g, ps):
                sk = psk.tile([128, b_core], F16, tag=f"sk{d}{g}",
                              name=f"sk{d}{g}")
                # PSUM readers are limited to ACT/DVE; alternate them.
                nc.scalar.copy(sk[:, 0:BT], ps[0][:])
                nc.vector.tensor_copy(sk[:, BT:2 * BT], ps[1][:])
                sks[(d, g)] = sk

            def zsl(d, g, i, fill):
                zt, off = zts[(d, g)]
                return zt[0:fill, off + i, :]

            def xsl(ci, fill, h):
                xt, slot = xsup[ci]
                return xt[0:fill, slot, BT * h:BT * (h + 1)]

            def stage1(d, g, di):
                items = plan[d][g]
                # h0/h1 use separate tags so they sit in different PSUM bank
                # pairs: alternating between banks of the same pair stalls PE
                ps = [ps_sk.tile([128, BT], F32, tag=f"skh{h}", name=f"ps{h}")
                      for h in range(NBT)]
                for i, (ci, zi) in enumerate(items):
                    fill = fills[ci]
                    st = (i == 0)
                    sp = (i == len(items) - 1)
                    for h in range(NBT):
                        nc.tensor.matmul(
                            ps[h][:], zsl(d, g, i, fill), xsl(ci, fill, h),
                            start=st, stop=sp)
                s1_drain(d, g, ps)

            def stage1_quad(gpair):
                """Degrees 0 and 1 of two block-groups run as eight
                interleaved accumulation chains across all 8 PSUM banks, so
                early-arriving X chunks enable 8x the PE work while stage 1
                races the X stream."""
                pstiles = {}
                for g in gpair:
                    pstiles[(0, g, 0)] = ps_sk.tile(
                        [128, BT], F32, tag="skh0", name=f"pa{g}0")
                    pstiles[(0, g, 1)] = ps_sk.tile(
                        [128, BT], F32, tag="skh1", name=f"pa{g}1")
                    pstiles[(1, g, 0)] = ps_fr.tile(
                        [128, BT], F32, tag="re", name=f"pb{g}0")
                    pstiles[(1, g, 1)] = ps_fr.tile(
                        [128, BT], F32, tag="im", name=f"pb{g}1")
                n = max(len(plan[d][g]) for d in (0, 1) for g in gpair)
                for i in range(n):
                    for g in gpair:
                        for d in (0, 1):
                            items = plan[d][g]
                            if i >= len(items):
                                continue
                            ci, zi = items[i]
                            fill = fills[ci]
                            st = (i == 0)
                            sp = (i == len(items) - 1)
                            for h in range(NBT):
                                nc.tensor.matmul(
                                    pstiles[(d, g, h)][:],
                                    zsl(d, g, i, fill), xsl(ci, fill, h),
                                    start=st, stop=sp)
                for g in gpair:
                    for d in (0, 1):
                        s1_drain(d, g, [pstiles[(d, g, 0)],
                                        pstiles[(d, g, 1)]])

            # ---- stage 2+3 for one (degree, half, pi): 8 matmuls -> fp16
            # cast -> complex product chain into prod[h].
            prods = {}
            rowfix = {}

            def get_prod(h):
                if h not in prods:
                    prods[h] = pprod.tile([128, 4, BT], F16, tag=f"prod{h}",
                                          name=f"prod{h}")
                return prods[h]

            def stage23(d, h, pi):
                prod = get_prod(h)
                mre = 128 * pi
                mim = 257 + 128 * pi
                frre = ps_fr.tile([128, BT], F32, tag="re")
                frim = ps_fr.tile([128, BT], F32, tag="im")
                # re/im interleaved so consecutive matmuls alternate banks
                for q in range(4):
                    nc.tensor.matmul(
                        frre[:], dftt[:, q, mre:mre + 128],
                        sks[(d, q)][:, BT * h:BT * (h + 1)],
                        start=(q == 0), stop=(q == 3))
                    nc.tensor.matmul(
                        frim[:], dftt[:, q, mim:mim + 128],
                        sks[(d, q)][:, BT * h:BT * (h + 1)],
                        start=(q == 0), stop=(q == 3))
                pre = prod[:, pi, :]
                pim = prod[:, 2 + pi, :]
                if d == 0:
                    # cast PSUM f32 -> fp16 directly into prod (ACT engine;
                    # GPSIMD cannot read PSUM)
                    nc.scalar.copy(pre, frre[:])
                    nc.scalar.copy(pim, frim[:])
                    if pi == 0:
                        dc = prow.tile([1, BT], F16, tag=f"dc{h}")
                        ny = prow.tile([1, BT], F16, tag=f"ny{h}")
                        nc.scalar.copy(dc[:], frre[0:1, :])
                        nc.scalar.copy(ny[:], frim[0:1, :])
                        rowfix[h] = (dc, ny)
                else:
                    fre = pfr.tile([128, BT], F16, tag="fre")
                    fim = pfr.tile([128, BT], F16, tag="fim")
                    nc.scalar.copy(fre[:], frre[:])
                    nc.scalar.copy(fim[:], frim[:])
                    if pi == 0:
                        dc0, ny0 = rowfix[h]
                        dc = prow.tile([1, BT], F16, tag=f"dc{h}")
                        ny = prow.tile([1, BT], F16, tag=f"ny{h}")
                        nc.vector.tensor_mul(dc[:], dc0[:], fre[0:1, :])
                        nc.vector.tensor_mul(ny[:], ny0[:], fim[0:1, :])
                        rowfix[h] = (dc, ny)
                    # all on DVE: GPSIMD is ~4x slower per op and was the
                    # critical path; 48 fp16 ops here are ~18us on DVE total
                    t1 = ptmp.tile([128, BT], F16, tag="t1")
                    t2 = ptmp.tile([128, BT], F16, tag="t2")
                    t3 = ptmp.tile([128, BT], F16, tag="t3")
                    t4 = ptmp.tile([128, BT], F16, tag="t4")
                    nc.vector.tensor_mul(t1[:], pre, fre[:])
                    nc.vector.tensor_mul(t2[:], pim, fim[:])
                    nc.vector.tensor_mul(t3[:], pre, fim[:])
                    nc.vector.tensor_mul(t4[:], pim, fre[:])
                    nc.vector.tensor_sub(pre, t1[:], t2[:])
                    nc.vector.tensor_add(pim, t3[:], t4[:])
                    if d == DEG - 1 and pi == 0:
                        dc, ny = rowfix[h]
                        nc.vector.tensor_copy(prod[0:1, 0, :], dc[:])
                        nc.vector.tensor_copy(prod[0:1, 2, :], ny[:])

            # ---- stage 4: irfft as matmul, drain via SBUF f32, DMA out.
            def stage4(h):
                prod = prods[h]
                for mp in ((0, 1), (2, 3)):
                    pos = {}
                    for m in mp:
                        pos[m] = ps_sk.tile([128, BT], F32, tag=f"skh{m % 2}",
                                            name=f"po{m}")
                    # m-pair interleaved so consecutive matmuls alternate banks
                    for q in range(4):
                        for m in mp:
                            nc.tensor.matmul(
                                pos[m][:], icot[:, q, 128 * m:128 * (m + 1)],
                                prod[:, q, :],
                                start=(q == 0), stop=(q == 3))
                    for m in mp:
                        ob = pout.tile([128, BT], F32, tag=f"ob{m % 2}")
                        if m % 2 == 0:
                            nc.scalar.copy(ob[:], pos[m][:])
                        else:
                            nc.vector.tensor_copy(ob[:], pos[m][:])
                        nc.scalar.dma_start(
                            ot[128 * m:128 * (m + 1), BT * h:BT * (h + 1)],
                            ob[:])

            # ---- emission schedule: h0 stage-1 passes first (they only need
            # half of X), with the h0 DFT/product chains woven in as each
            # degree completes; h1 passes follow while h1 X streams in.
            di = 0
            stage1_quad((0, 1))
            stage1_quad((2, 3))
            for pi in range(2):
                stage23(0, 0, pi)
            stage1(2, 0, di); di += 1
            stage1(2, 1, di); di += 1
            for pi in range(2):
                stage23(0, 1, pi)
            stage1(2, 2, di); di += 1
            stage1(2, 3, di); di += 1
            for pi in range(2):
                stage23(1, 0, pi)
            for pi in range(2):
                stage23(1, 1, pi)
            for pi in range(2):
                stage23(2, 0, pi)
            for pi in range(2):
                stage23(2, 1, pi)
            stage4(0)
            stage4(1)

    nc.compile()
    return nc


def round_fp16(x):
    return np.asarray(x, np.float32).astype(np.float16)


def prepare_inputs(X, index_hash, bit_hash):
    order, chunks, plan, zm_t = build_plan(index_hash, bit_hash)
    dft_t, ico_t = build_dft_tables()
    nmm = zm_t.shape[1]
    # padded layout: chunk c occupies rows [128c, 128c+fill), rest zero
    Xt = np.asarray(X, np.float32).T[order]
    Xp = np.zeros((len(chunks) * 128, Xt.shape[1]), np.float16)
    for c, (start, fill) in enumerate(chunks):
        Xp[128 * c:128 * c + fill] = Xt[start:start + fill]
    shared = {
        "zm": round_fp16(zm_t),          # +-1/0: exact in fp16
        "dft": round_fp16(dft_t),
        "ico": round_fp16(ico_t),
    }
    return plan, chunks, nmm, Xp, shared


def kernel(X, index_hash, bit_hash, _trace=False):
    plan, chunks, nmm, Xp, shared = prepare_inputs(X, index_hash, bit_hash)
    nc = build_program(plan, chunks, nmm)
    in_maps = [
        {"xp": np.ascontiguousarray(Xp[:, c * B_CORE:(c + 1) * B_CORE]), **shared}
        for c in range(NCORES)
    ]
    res = bass_utils.run_bass_kernel_spmd(
        nc, in_maps, core_ids=list(range(NCORES)), trace=_trace)
    out = np.empty((B, NCOMP), np.float32)
    for c in range(NCORES):
        out[c * B_CORE:(c + 1) * B_CORE] = res.results[c]["ot"].T
    return (out, res) if _trace else out


# revision 55
# speedup vs baseline: 1.0289x; 1.0289x over previous
"""PolyCntSketch (TensorSketch, degree 3) Trainium2 kernel.

Math: for each degree d, CountSketch_d = X @ S_d (S_d one-hot signed), then
out = irfft(prod_d rfft(CountSketch_d)).

Device strategy (pure data parallelism over batch, 8 cores, B_core = 1024):
  - Host feeds X transposed ([F, B_core]) in fp16, features packed into
    128-row chunks where each chunk holds whole (block_d0, block_d1,
    block_d2)-classes (block = idx_d // 128), so each chunk touches few
    128-bucket blocks per degree -> few segment matmuls.
  - Stage 1 (weight-stationary, full batch): per (degree, block) the plan's
    Z one-hot matrices accumulate sketch PSUM [128, 512] x 2 half-batches.
    PSUM drains to fp16 SBUF via scalar/gpsimd/vector round-robin.
  - Stage 2: rfft as DFT matmul (512 -> 257 complex), fp16 weights. The
    Nyquist bin Re(256) rides in the identically-zero Im(0) column of the
    DFT matrix (patched to the alternating +-1 column), so it needs no
    extra matmuls; the DC and Nyquist rows of the complex product are
    fixed up with [1, 512] ops at the end of each degree chain.
  - Stage 3: complex product across the 3 degrees, all fp16 (2x DVE rate).
    DFT is scaled by 1/32 (exact) so the fp16 product cannot overflow;
    the irfft table is scaled by 2^15 to compensate.
  - Stage 4: irfft as matmul -> out^T [512, B_core] f32.
"""
import sys

for _p in ("/opt/trn_rl_repo",):
    if _p not in sys.path:
        sys.path.append(_p)

import numpy as np

from concourse import bacc, mybir, tile
from concourse import bass_utils

F16 = mybir.dt.float16
F32 = mybir.dt.float32

B, F, NCOMP, DEG = 8192, 4096, 512, 3
NCORES = 8
B_CORE = B // NCORES
BT = 512                     # batch columns per matmul (PSUM bank = 512 f32)
NBT = B_CORE // BT           # 2 half-batches
CHUNK = 128
NBLK = NCOMP // 128          # 4 bucket blocks
DFT_SCALE = 1.0 / 32.0       # exact power of two; keeps fp16 products small


def _pack_classes(kvs_counts, seed_count=8, iters=60000):
    """Pack (g0,g1,g2)-classes into <=128-row bins, minimizing the total
    matmul count sum_bins sum_d #distinct-blocks. Greedy first-fit by
    marginal cost + hill climbing with move/swap steps."""
    import random

    def cost_of(binkvs):
        if not binkvs:
            return 0
        return sum(len(set((kv >> sh) & 3 for kv in binkvs))
                   for sh in (4, 2, 0))

    items = []
    for kv, s in kvs_counts:
        while s > CHUNK:
            items.append((kv, CHUNK)); s -= CHUNK
        if s:
            items.append((kv, s))

    def greedy(order_classes):
        bins, sizes = [], []
        for kv, s in order_classes:
            best, bestdelta = None, None
            for i, b in enumerate(bins):
                if sizes[i] + s <= CHUNK:
                    delta = (cost_of([k for k, _ in b] + [kv])
                             - cost_of([k for k, _ in b]))
                    if bestdelta is None or delta < bestdelta:
                        best, bestdelta = i, delta
            if best is not None and bestdelta <= 1:
                bins[best].append((kv, s)); sizes[best] += s
            else:
                bins.append([(kv, s)]); sizes.append(s)
        return bins, sizes

    def hill(bins, sizes, seed):
        rng = random.Random(seed)
        bins = [list(b) for b in bins]; sizes = list(sizes)

        def bc(i):
            return cost_of([k for k, _ in bins[i]])

        for _ in range(iters):
            r = rng.random()
            i = rng.randrange(len(bins)); j = rng.randrange(len(bins))
            if i == j or not bins[i]:
                continue
            if r < 0.6:
                ii = rng.randrange(len(bins[i])); kv, s = bins[i][ii]
                if sizes[j] + s > CHUNK:
                    continue
                cb = bc(i) + bc(j)
                bi2 = [x for xi, x in enumerate(bins[i]) if xi != ii]
                ca = (cost_of([k for k, _ in bi2])
                      + cost_of([k for k, _ in bins[j]] + [kv]))
                if ca <= cb:
                    bins[i].pop(ii); bins[j].append((kv, s))
                    sizes[i] -= s; sizes[j] += s
            else:
                if not bins[j]:
                    continue
                ii = rng.randrange(len(bins[i])); jj = rng.randrange(len(bins[j]))
                kv1, s1 = bins[i][ii]; kv2, s2 = bins[j][jj]
                if sizes[i] - s1 + s2 > CHUNK or sizes[j] - s2 + s1 > CHUNK:
                    continue
                cb = bc(i) + bc(j)
                bi2 = [x for xi, x in enumerate(bins[i]) if xi != ii] + [(kv2, s2)]
                bj2 = [x for xj, x in enumerate(bins[j]) if xj != jj] + [(kv1, s1)]
                ca = (cost_of([k for k, _ in bi2])
                      + cost_of([k for k, _ in bj2]))
                if ca <= cb:
                    bins[i][ii] = (kv2, s2); bins[j][jj] = (kv1, s1)
                    sizes[i] += s2 - s1; sizes[j] += s1 - s2
        bins = [b for b in bins if b]
        return bins, sum(cost_of([k for k, _ in b]) for b in bins)

    best = None
    for seed in range(seed_count):
        o = items[:]
        random.Random(seed).shuffle(o)
        if seed % 2 == 0:
            o.sort(key=lambda x: (x[0] >> 2,))
        bins, sizes = greedy(o)
        bins, c = hill(bins, sizes, seed)
        if best is None or c < best[0]:
            best = (c, [list(b) for b in bins])
    return best[1]


def build_plan(index_hash, bit_hash):
    """Pack whole (g0,g1,g2)-classes into 128-row chunks minimizing the
    count of per-(chunk, degree, block) matmuls.

    Returns:
      order [F]: feature order for the transposed X upload
      chunks: list of (start, fill) row ranges into the ordered X
      plan[d][g]: list of (chunk_idx, zslot) in emission order ((d,g)-major)
      zm_t [128, nmm, 128]: stacked Z matrices, partition-major
    """
    idx = np.asarray(index_hash)
    sgn = (np.asarray(bit_hash) * 2 - 1).astype(np.float32)
    blocks = idx >> 7
    key = blocks[0] * 16 + blocks[1] * 4 + blocks[2]

    kvs, counts = np.unique(key, return_counts=True)
    bins = _pack_classes(sorted(zip(kvs.tolist(), counts.tolist())))

    # features per class, consumed front-to-back as bins reference (possibly
    # split) classes
    feat_of = {int(kv): np.nonzero(key == kv)[0].tolist() for kv in kvs}
    order = []
    chunks = []
    for b in bins:
        start = len(order)
        for kv, s in b:
            take = feat_of[kv][:s]
            feat_of[kv] = feat_of[kv][s:]
            order.extend(take)
        chunks.append((start, len(order) - start))
    order = np.array(order)
    assert len(order) == F and len(np.unique(order)) == F

    items = [[[] for _ in range(NBLK)] for _ in range(DEG)]
    for ci, (start, fill) in enumerate(chunks):
        feats = order[start:start + fill]
        for d in range(DEG):
            for g in np.unique(blocks[d, feats]):
                g = int(g)
                rows = np.nonzero(blocks[d, feats] == g)[0]
                Z = np.zeros((CHUNK, 128), np.float32)
                Z[rows, idx[d, feats[rows]] - 128 * g] = sgn[d, feats[rows]]
                items[d][g].append((ci, Z))
    for d in range(DEG):
        for g in range(NBLK):
            if not items[d][g]:
                items[d][g].append((0, np.zeros((CHUNK, 128), np.float32)))

    zmats = []
    plan = [[[] for _ in range(NBLK)] for _ in range(DEG)]
    for d in range(DEG):
        for g in range(NBLK):
            for (ci, Z) in sorted(items[d][g], key=lambda x: x[0]):
                plan[d][g].append((ci, len(zmats)))
                zmats.append(Z)

    # permute chunks into first-use order of the stage-1 quad schedule so
    # consumption-ordered super-tile DMAs read contiguous DRAM rows
    nch = len(chunks)
    seen = set()
    use = []
    for gpair in ((0, 1), (2, 3)):
        n = max(len(plan[d][g]) for d in (0, 1) for g in gpair)
        for i in range(n):
            for g in gpair:
                for d in (0, 1):
                    if i < len(plan[d][g]):
                        ci = plan[d][g][i][0]
                        if ci not in seen:
                            seen.add(ci)
                            use.append(ci)
    for ci in range(nch):
        if ci not in seen:
            use.append(ci)
    old2new = {old: new for new, old in enumerate(use)}
    chunks = [chunks[old] for old in use]
    for d in range(DEG):
        for g in range(NBLK):
            plan[d][g] = [(old2new[ci], zi) for (ci, zi) in plan[d][g]]

    zm = np.stack(zmats)                                # [nmm, 128, 128]
    zm_t = np.ascontiguousarray(zm.transpose(1, 0, 2))  # [128, nmm, 128]
    return order, chunks, plan, zm_t


def build_dft_tables():
    n = np.arange(NCOMP)[:, None]
    k = np.arange(257)[None, :]
    ang = 2 * np.pi * n * k / NCOMP
    # stage-2 lhsT [512, 514]: cols 0..256 Re coeffs, cols 257..513 Im coeffs.
    # Col 257 is Im(0) == 0: replace it with the Nyquist column (-1)^n so
    # Re(256) rides in the Im(0) slot for free.
    dft = np.concatenate([np.cos(ang), -np.sin(ang)], axis=1)
    dft[:, 257] = np.cos(np.pi * np.arange(NCOMP))
    dft = (dft * DFT_SCALE).astype(np.float32)
    dft_t = np.ascontiguousarray(
        dft.reshape(4, 128, 514).transpose(1, 0, 2))    # [128, 4, 514]

    kk = np.arange(257)[:, None]
    nn = np.arange(NCOMP)[None, :]
    ang2 = 2 * np.pi * kk * nn / NCOMP
    ck = np.full((257, 1), 2.0, np.float32); ck[0] = 1.0
    dk = np.full((257, 1), 2.0, np.float32); dk[0] = 0.0
    iscale = 1.0 / DFT_SCALE ** 3
    ire = (ck * np.cos(ang2) / NCOMP) * iscale           # [257, 512]
    iim = (-dk * np.sin(ang2) / NCOMP) * iscale
    ico = np.zeros((4, 128, NCOMP), np.float32)
    ico[0] = ire[0:128]
    ico[1] = ire[128:256]
    ico[2] = iim[0:128]
    ico[3] = iim[128:256]
    # Nyquist rides in the Im(0) slot; its irfft column is (1/N)(-1)^n.
    ico[2, 0] = np.cos(np.pi * np.arange(NCOMP)) / NCOMP * iscale
    ico_t = np.ascontiguousarray(ico.transpose(1, 0, 2))   # [128, 4, 512]
    return dft_t, ico_t


def build_program(plan, chunks, nmm, b_core=B_CORE):
    nch = len(chunks)
    fills = [f for (_, f) in chunks]
    # (d, g) -> [lo, hi) slice into the z stack
    zoff = {}
    pos = 0
    for d in range(DEG):
        for g in range(NBLK):
            zoff[(d, g)] = (pos, pos + len(plan[d][g]))
            pos += len(plan[d][g])
    assert pos == nmm

    nc = bacc.Bacc("TRN2", target_bir_lowering=False, debug=False)
    xp = nc.dram_tensor("xp", [nch * 128, b_core], F16,
                        kind="ExternalInput").ap()
    zm = nc.dram_tensor("zm", [128, nmm, 128], F16, kind="ExternalInput").ap()
    dft = nc.dram_tensor("dft", [128, 4, 514], F16, kind="ExternalInput").ap()
    ico = nc.dram_tensor("ico", [128, 4, 512], F16, kind="ExternalInput").ap()
    ot = nc.dram_tensor("ot", [NCOMP, b_core], F32, kind="ExternalOutput").ap()

    with tile.TileContext(nc) as tc:
        with (
            tc.tile_pool(name="pz", bufs=1) as pz,
            tc.tile_pool(name="pc", bufs=1) as pc,
            tc.tile_pool(name="px", bufs=1) as px,
            tc.tile_pool(name="psk", bufs=1) as psk,
            tc.tile_pool(name="pprod", bufs=1) as pprod,
            tc.tile_pool(name="pfr", bufs=2) as pfr,
            tc.tile_pool(name="ptmp", bufs=2) as ptmp,
            tc.tile_pool(name="prow", bufs=2) as prow,
            tc.tile_pool(name="pout", bufs=2) as pout,
            tc.tile_pool(name="ps_sk", bufs=2, space="PSUM") as ps_sk,
            tc.tile_pool(name="ps_fr", bufs=2, space="PSUM") as ps_fr,
        ):
            # ---- DMA: each dma_start costs ~0.6us of issue time on the
            # sync sequencer, so batch aggressively: X chunks load in
            # consumption-ordered super-tiles (up to 4 chunks = 1MB per
            # issue; the first ones smaller so PE starts ASAP), and the z
            # stacks merge to one issue per (degree, group-pair).
            xsup = {}    # ci -> (tile, slot)
            zts = {}     # (d, g) -> (tile, item offset)

            def load_x(ci):
                xt = px.tile([128, 1, b_core], F16, tag=f"x{ci}",
                             name=f"x{ci}")
                nc.sync.dma_start(xt[:, 0, :], xp[128 * ci:128 * (ci + 1), :])
                xsup[ci] = (xt, 0)

            def load_zmerge(d, glo, ghi, name, split_first=False, eng=None):
                eng = eng or nc.sync
                lo = zoff[(d, glo)][0]
                hi = zoff[(d, ghi)][1]
                zt = pz.tile([128, hi - lo, 128], F16, tag=name, name=name)
                for g in range(glo, ghi + 1):
                    zts[(d, g)] = (zt, zoff[(d, g)][0] - lo)
                if split_first:
                    eng.dma_start(zt[:, 0:1, :], zm[:, lo:lo + 1, :])
                    eng.dma_start(zt[:, 1:, :], zm[:, lo + 1:hi, :])
                else:
                    eng.dma_start(zt[:], zm[:, lo:hi, :])

            # z tables issue on the ACT queue, X chunks on the sync queue:
            # two parallel issue pipes (~0.6us per dma_start issue). Only
            # phase-A z bodies go up front; the phase-B/degree-2 tables are
            # issued later (in the emission section) so their transfers
            # don't steal DMA bandwidth from the X stream.
            load_zmerge(0, 0, 0, "z00", split_first=True, eng=nc.scalar)
            load_zmerge(1, 0, 0, "z10", split_first=True, eng=nc.scalar)
            load_zmerge(0, 1, 1, "z01", split_first=True, eng=nc.scalar)
            load_zmerge(1, 1, 1, "z11", split_first=True, eng=nc.scalar)
            for ci in range(min(21, nch)):
                load_x(ci)
            load_zmerge(0, 2, 3, "zB0")
            load_zmerge(1, 2, 3, "zB1")
            for ci in range(21, nch):
                load_x(ci)
            dftt = pc.tile([128, 4, 514], F16, tag="dftt")
            nc.sync.dma_start(dftt[:], dft[:])
            load_zmerge(2, 0, 3, "zC")
            icot = pc.tile([128, 4, 512], F16, tag="icot")
            nc.sync.dma_start(icot[:], ico[:])

            # ---- stage 1: weight-stationary count sketch over the full
            # batch; the two batch-halves interleave per item so consecutive
            # matmuls alternate PSUM banks (same-bank back-to-back
            # accumulation stalls PE).
            sks = {}

            def s1_drain(d, g, ps):
                sk = psk.tile([128, b_core], F16, tag=f"sk{d}{g}",
                              name=f"sk{d}{g}")
                # PSUM readers are limited to ACT/DVE; alternate them.
                nc.scalar.copy(sk[:, 0:BT], ps[0][:])
                nc.vector.tensor_copy(sk[:, BT:2 * BT], ps[1][:])
                sks[(d, g)] = sk

            def zsl(d, g, i, fill):
                zt, off = zts[(d, g)]
                return zt[0:fill, off + i, :]

            def xsl(ci, fill, h):
                xt, slot = xsup[ci]
                return xt[0:fill, slot, BT * h:BT * (h + 1)]

            def stage1(d, g, di):
                items = plan[d][g]
                # h0/h1 use separate tags so they sit in different PSUM bank
                # pairs: alternating between banks of the same pair stalls PE
                ps = [ps_sk.tile([128, BT], F32, tag=f"skh{h}", name=f"ps{h}")
                      for h in range(NBT)]
                for i, (ci, zi) in enumerate(items):
                    fill = fills[ci]
                    st = (i == 0)
                    sp = (i == len(items) - 1)
                    for h in range(NBT):
                        nc.tensor.matmul(
                            ps[h][:], zsl(d, g, i, fill), xsl(ci, fill, h),
                            start=st, stop=sp)
                s1_drain(d, g, ps)

            def stage1_quad(gpair):
                """Degrees 0 and 1 of two block-groups run as eight
                interleaved accumulation chains across all 8 PSUM banks, so
                early-arriving X chunks enable 8x the PE work while stage 1
                races the X stream."""
                pstiles = {}
                for g in gpair:
                    pstiles[(0, g, 0)] = ps_sk.tile(
                        [128, BT], F32, tag="skh0", name=f"pa{g}0")
                    pstiles[(0, g, 1)] = ps_sk.tile(
                        [128, BT], F32, tag="skh1", name=f"pa{g}1")
                    pstiles[(1, g, 0)] = ps_fr.tile(
                        [128, BT], F32, tag="re", name=f"pb{g}0")
                    pstiles[(1, g, 1)] = ps_fr.tile(
                        [128, BT], F32, tag="im", name=f"pb{g}1")
                n = max(len(plan[d][g]) for d in (0, 1) for g in gpair)
                for i in range(n):
                    for g in gpair:
                        for d in (0, 1):
                            items = plan[d][g]
                            if i >= len(items):
                                continue
                            ci, zi = items[i]
                            fill = fills[ci]
                            st = (i == 0)
                            sp = (i == len(items) - 1)
                            for h in range(NBT):
                                nc.tensor.matmul(
                                    pstiles[(d, g, h)][:],
                                    zsl(d, g, i, fill), xsl(ci, fill, h),
                                    start=st, stop=sp)
                for g in gpair:
                    for d in (0, 1):
                        s1_drain(d, g, [pstiles[(d, g, 0)],
                                        pstiles[(d, g, 1)]])

            # ---- stage 2+3 for one (degree, half, pi): 8 matmuls -> fp16
            # cast -> complex product chain into prod[h].
            prods = {}
            rowfix = {}

            def get_prod(h):
                if h not in prods:
                    prods[h] = pprod.tile([128, 4, BT], F16, tag=f"prod{h}",
                                          name=f"prod{h}")
                return prods[h]

            def stage23(d, h, pi):
                prod = get_prod(h)
                mre = 128 * pi
                mim = 257 + 128 * pi
                frre = ps_fr.tile([128, BT], F32, tag="re")
                frim = ps_fr.tile([128, BT], F32, tag="im")
                # re/im interleaved so consecutive matmuls alternate banks
                for q in range(4):
                    nc.tensor.matmul(
                        frre[:], dftt[:, q, mre:mre + 128],
                        sks[(d, q)][:, BT * h:BT * (h + 1)],
                        start=(q == 0), stop=(q == 3))
                    nc.tensor.matmul(
                        frim[:], dftt[:, q, mim:mim + 128],
                        sks[(d, q)][:, BT * h:BT * (h + 1)],
                        start=(q == 0), stop=(q == 3))
                pre = prod[:, pi, :]
                pim = prod[:, 2 + pi, :]
                if d == 0:
                    # cast PSUM f32 -> fp16 directly into prod (ACT engine;
                    # GPSIMD cannot read PSUM)
                    nc.scalar.copy(pre, frre[:])
                    nc.scalar.copy(pim, frim[:])
                    if pi == 0:
                        dc = prow.tile([1, BT], F16, tag=f"dc{h}")
                        ny = prow.tile([1, BT], F16, tag=f"ny{h}")
                        nc.scalar.copy(dc[:], frre[0:1, :])
                        nc.scalar.copy(ny[:], frim[0:1, :])
                        rowfix[h] = (dc, ny)
                else:
                    fre = pfr.tile([128, BT], F16, tag="fre")
                    fim = pfr.tile([128, BT], F16, tag="fim")
                    nc.scalar.copy(fre[:], frre[:])
                    nc.scalar.copy(fim[:], frim[:])
                    if pi == 0:
                        dc0, ny0 = rowfix[h]
                        dc = prow.tile([1, BT], F16, tag=f"dc{h}")
                        ny = prow.tile([1, BT], F16, tag=f"ny{h}")
                        nc.vector.tensor_mul(dc[:], dc0[:], fre[0:1, :])
                        nc.vector.tensor_mul(ny[:], ny0[:], fim[0:1, :])
                        rowfix[h] = (dc, ny)
                    # all on DVE: GPSIMD is ~4x slower per op and was the
                    # critical path; 48 fp16 ops here are ~18us on DVE total
                    t1 = ptmp.tile([128, BT], F16, tag="t1")
                    t2 = ptmp.tile([128, BT], F16, tag="t2")
                    t3 = ptmp.tile([128, BT], F16, tag="t3")
                    t4 = ptmp.tile([128, BT], F16, tag="t4")
                    nc.vector.tensor_mul(t1[:], pre, fre[:])
                    nc.vector.tensor_mul(t2[:], pim, fim[:])
                    nc.vector.tensor_mul(t3[:], pre, fim[:])
                    nc.vector.tensor_mul(t4[:], pim, fre[:])
                    nc.vector.tensor_sub(pre, t1[:], t2[:])
                    nc.vector.tensor_add(pim, t3[:], t4[:])
                    if d == DEG - 1 and pi == 0:
                        dc, ny = rowfix[h]
                        nc.vector.tensor_copy(prod[0:1, 0, :], dc[:])
                        nc.vector.tensor_copy(prod[0:1, 2, :], ny[:])

            # ---- stage 4: irfft as matmul, drain via SBUF f32, DMA out.
            def stage4(h):
                prod = prods[h]
                for mp in ((0, 1), (2, 3)):
                    pos = {}
                    for m in mp:
                        pos[m] = ps_sk.tile([128, BT], F32, tag=f"skh{m % 2}",
                                            name=f"po{m}")
                    # m-pair interleaved so consecutive matmuls alternate banks
                    for q in range(4):
                        for m in mp:
                            nc.tensor.matmul(
                                pos[m][:], icot[:, q, 128 * m:128 * (m + 1)],
                                prod[:, q, :],
                                start=(q == 0), stop=(q == 3))
                    for m in mp:
                        ob = pout.tile([128, BT], F32, tag=f"ob{m % 2}")
                        if m % 2 == 0:
                            nc.scalar.copy(ob[:], pos[m][:])
                        else:
                            nc.vector.tensor_copy(ob[:], pos[m][:])
                        nc.scalar.dma_start(
                            ot[128 * m:128 * (m + 1), BT * h:BT * (h + 1)],
                            ob[:])

            # ---- emission schedule: h0 stage-1 passes first (they only need
            # half of X), with the h0 DFT/product chains woven in as each
            # degree completes; h1 passes follow while h1 X streams in.
            di = 0
            stage1_quad((0, 1))
            stage1_quad((2, 3))
            for pi in range(2):
                stage23(0, 0, pi)
            stage1(2, 0, di); di += 1
            stage1(2, 1, di); di += 1
            for pi in range(2):
                stage23(0, 1, pi)
            stage1(2, 2, di); di += 1
            stage1(2, 3, di); di += 1
            for pi in range(2):
                stage23(1, 0, pi)
            for pi in range(2):
                stage23(1, 1, pi)
            for pi in range(2):
                stage23(2, 0, pi)
            for pi in range(2):
                stage23(2, 1, pi)
            stage4(0)
            stage4(1)

    nc.compile()
    return nc


def round_fp16(x):
    return np.asarray(x, np.float32).astype(np.float16)


def prepare_inputs(X, index_hash, bit_hash):
    order, chunks, plan, zm_t = build_plan(index_hash, bit_hash)
    dft_t, ico_t = build_dft_tables()
    nmm = zm_t.shape[1]
    # padded layout: chunk c occupies rows [128c, 128c+fill), rest zero
    Xt = np.asarray(X, np.float32).T[order]
    Xp = np.zeros((len(chunks) * 128, Xt.shape[1]), np.float16)
    for c, (start, fill) in enumerate(chunks):
        Xp[128 * c:128 * c + fill] = Xt[start:start + fill]
    shared = {
        "zm": round_fp16(zm_t),          # +-1/0: exact in fp16
        "dft": round_fp16(dft_t),
        "ico": round_fp16(ico_t),
    }
    return plan, chunks, nmm, Xp, shared


def kernel(X, index_hash, bit_hash, _trace=False):
    plan, chunks, nmm, Xp, shared = prepare_inputs(X, index_hash, bit_hash)
    nc = build_program(plan, chunks, nmm)
    in_maps = [
        {"xp": np.ascontiguousarray(Xp[:, c * B_CORE:(c + 1) * B_CORE]), **shared}
        for c in range(NCORES)
    ]
    res = bass_utils.run_bass_kernel_spmd(
        nc, in_maps, core_ids=list(range(NCORES)), trace=_trace)
    out = np.empty((B, NCOMP), np.float32)
    for c in range(NCORES):
        out[c * B_CORE:(c + 1) * B_CORE] = res.results[c]["ot"].T
    return (out, res) if _trace else out
